# revision 21
# baseline (speedup 1.0000x reference)
"""XNOR-Net BasicBlock forward on 8 Trainium2 NeuronCores.

Pipeline (training-mode BN => sync-BN over the batch):
    bn1(x)+m1 -> sign -> gconv3x3(256->128, g=2) -> prelu(a1)
    -> bn2+m2 -> sign -> gconv3x3(128->256, g=2) -> prelu(a2)
    -> bn3 + x -> +m3 -> prelu(a3) -> +m4

Sharding: data-parallel over batch (16 imgs -> 2 per core). BN batch
stats are synchronized with three tiny AllReduces of per-channel
moment vectors ([128, 4..8] f32).

Binarization exactness: sign(act) in {-1,0,1} and sign(w) in {-1,1}
are exact in bf16, products are exact, and PSUM accumulation is f32
(sums bounded by 1152) -- so the convs are computed exactly; the
per-out-channel scale factors sf = mean|w| are applied afterwards in
f32 (or folded into downstream affine coefficients).

Conv as matmul: activations are stored channel-on-partition in a
zero-padded [58 x 58] per-image spatial layout, so each of the 9
kernel taps is a contiguous shifted slice of the flat padded buffer;
conv = 9 PSUM-accumulated matmuls. Padded border positions compute
garbage which is never read (stats/sign/final ops read interior APs).
"""

import numpy as np

from concourse import bass, mybir, tile
from concourse.tile import ScopedClock

F32 = mybir.dt.float32
BF16 = mybir.dt.bfloat16
AF = mybir.ActivationFunctionType
OP = mybir.AluOpType

NCORES = 8
PER = 2            # images per core
C = 256            # channels in/out
CM = 128           # mid channels (conv1 out)
H = 56
HP = H + 2         # padded spatial
IMG = H * H        # 3136
IMGP = HP * HP     # 3364
F = PER * IMG      # 6272 valid positions per core
FP = PER * IMGP    # 6728 padded positions per core
MARG = 64          # flat guard margin around padded buffers
EPS = 1e-5

# conv chunking: row-aligned blocks of 8 padded rows (464 <= 512 PSUM
# bank), so PSUM eviction can write only interior values to contiguous
# unpadded buffers. (image, block, flat_start, flat_len)
CHUNKS = [
    (n, b, n * IMGP + b * 8 * HP, (8 if b < 7 else 2) * HP)
    for n in range(PER)
    for b in range(8)
]


def _evict_rows(b):
    # -> (first valid row in psum view, n rows, first output row)
    if b == 0:
        return 1, 7, 0
    if b == 7:
        return 0, 1, 55
    return 0, 8, 8 * b - 1

_PATCHED = False


def _apply_tile_patch():
    """This walrus build rejects >1 sync-wait per CTRL instruction and
    any sem-eq wait on Drain. Replace TileContext's tail butterfly
    barrier with a ge-only equivalent and split the tail drain's waits
    across single-wait drain instructions."""
    global _PATCHED
    if _PATCHED:
        return
    _PATCHED = True
    import bass_rust
    from concourse.bass import compact_to_ranges

    def _drain_and_barrier_ge(self, tick_clock, wait_clock):
        nc = self.nc
        drain_inst = nc.sync.drain()
        wait_clock.add_sem_waits(
            drain_inst.ins, ScopedClock({None: tick_clock.global_clock})
        )
        waits = list(drain_inst.ins.sync_info.on_wait)
        if len(waits) > 1:
            drain_inst.ins.sync_info.on_wait = waits[:1]
            for w in waits[1:]:
                d = nc.sync.drain()
                d.ins.sync_info = bass_rust.SyncInfo(on_wait=[w], on_update=[])

        bar = nc.alloc_semaphore(f"tail_bar_{nc.next_id()}")
        n_eng = len(nc.engines)
        for eng in nc.engines.values():
            eng.drain(fusable=False).then_inc(bar, 1)
        for eng in nc.engines.values():
            eng.wait_ge(bar, n_eng)

        popped = nc._tile_sem_poison_stack.pop()
        assert popped is self._sem_poison
        assert self.sems is not None
        sems = list(self.sems.allocated().values())
        sem_nums = [s.num if hasattr(s, "num") else s for s in sems]
        sem_nums.append(bar.num)
        for rng in compact_to_ranges(sorted(set(sem_nums))):
            nc.gpsimd.dma_reset(rng)
            nc.gpsimd.sem_clear(rng)

    tile.TileContext._drain_and_barrier = _drain_and_barrier_ge


_CTRL_TYPES = ("InstDrain", "InstNoOp", "InstEventSemaphore",
               "InstUnconditionalBranch", "InstHalt")


def _split_excess_waits(nc, maxw=1):
    """This walrus build encodes a limited number of sync-wait commands
    per instruction (1 for CTRL-class, ~2 for compute). Move excess
    waits onto same-engine NoOp carriers inserted just before."""
    import bass_rust

    uid = [0]
    for f in nc.m.functions:
        for bb in f.blocks:
            il = bb.instructions
            out = []
            changed = False
            for ins in il:
                si = getattr(ins, "sync_info", None)
                waits = list(si.on_wait) if si is not None else []
                lim = 1 if type(ins).__name__ in _CTRL_TYPES else maxw
                if len(waits) > lim:
                    keep = waits[-lim:]
                    excess = waits[:-lim]
                    ins.sync_info = bass_rust.SyncInfo(
                        on_wait=keep, on_update=list(si.on_update)
                    )
                    for i in range(0, len(excess), 1):
                        from concourse import mybir as _mb

                        nop = _mb.InstNoOp(
                            name=f"waitnop-{uid[0]}", ins=[], outs=[]
                        )
                        uid[0] += 1
                        nop.engine = ins.engine
                        nop.sync_info = bass_rust.SyncInfo(
                            on_wait=[excess[i]], on_update=[]
                        )
                        out.append(nop)
                    changed = True
                out.append(ins)
            if changed:
                bb.instructions = out


def _shift(k):
    dy, dx = divmod(k, 3)
    return (dy - 1) * HP + (dx - 1)


def _host_consts(ip):
    """Precompute all value-derived constants on the host (f32)."""
    w1 = ip["w1"].astype(np.float32)   # [128, 128, 3, 3]
    w2 = ip["w2"].astype(np.float32)   # [256, 64, 3, 3]
    sf1 = np.abs(w1).reshape(CM, -1).mean(axis=1)          # [128]
    sf2 = np.abs(w2).reshape(C, -1).mean(axis=1)           # [256]
    s_w1 = np.sign(w1)
    s_w2 = np.sign(w2)

    # conv1 weights, lhsT layout [ci(128 part), g, k, o(64)] bf16
    w1t = np.zeros((CM, 2, 9, 64), np.float32)
    for g in range(2):
        for k in range(9):
            dy, dx = divmod(k, 3)
            # out channel o_global = g*64+o uses input channels g*128..+128
            w1t[:, g, k, :] = s_w1[g * 64:(g + 1) * 64, :, dy, dx].T
    # conv2 weights, lhsT layout [128 part, k, o(128)] bf16:
    # partitions 0..63 hold group-1 (ci = p), 64..127 group-2 (ci = p-64),
    # so the two groups' K=64 matmuls occupy disjoint PE row-groups.
    w2t = np.zeros((128, 9, 128), np.float32)
    for g in range(2):
        for k in range(9):
            dy, dx = divmod(k, 3)
            w2t[g * 64:(g + 1) * 64, k, :] = s_w2[g * 128:(g + 1) * 128, :, dy, dx].T

    def col2(v):  # [256] -> [128, 2]
        return v.astype(np.float32).reshape(2, CM).T.copy()

    def col1(v):  # [128] -> [128, 1]
        return v.astype(np.float32).reshape(1, CM).T.copy()

    a1 = ip["a1"].astype(np.float32)   # [128]
    a2 = ip["a2"].astype(np.float32)   # [256]
    a3 = ip["a3"].astype(np.float32)   # [256]

    assert np.all(a1 > 0), "sign2 threshold trick needs a1 > 0"
    assert np.all(ip["g2"] > 0), "sign2 threshold trick needs g2 > 0"
    assert np.all(ip["g3"] > 0), "phase4 abs trick needs g3 > 0"
    assert np.all(a2 < 1) and np.all(a2 > -1), "phase4 needs |a2| < 1"
    assert np.all(a3 < 1), "phase4 abs trick needs a3 < 1"
    assert np.all(sf2 > 0)

    consts = {
        # packed misc consts, one [128, n] f32 table
        "g1": col2(ip["g1"]), "b1m1": col2(ip["b1"] + ip["m1"]),
        "g2": col1(ip["g2"]), "b2m2": col1(ip["b2"] + ip["m2"]),
        "g3": col2(ip["g3"]), "b3m3": col2(ip["b3"] + ip["m3"]),
        "sf1": col1(sf1),
        "a1": col1(a1), "oma1": col1(1.0 - a1),
        "a1sq": col1(a1 * a1), "oma1sq": col1(1.0 - a1 * a1),
        "inva1": col1(1.0 / a1),
        # bn3 stat combination uses out2 = (1-a2)*sf2*r2 + a2*sf2*u2
        "casf": col2(a2 * sf2), "cbsf": col2((1.0 - a2) * sf2),
        "casq": col2(a2 * a2 * sf2 * sf2),
        "cbsq": col2((1.0 - a2 * a2) * sf2 * sf2),
        # phase4: out2 = h1p*u2 + h1m*|u2| (sf2 folded)
        "h1p": col2((1.0 + a2) * 0.5 * sf2),
        "h1m": col2((1.0 - a2) * 0.5 * sf2),
        "A3": col2((1.0 + a3) * 0.5), "B3": col2((1.0 - a3) * 0.5),
        "m4": col2(ip["m4"]),
    }
    m4_nonzero = bool(np.any(ip["m4"] != 0))
    return w1t, w2t, consts, m4_nonzero


def _build(w1t, w2t, consts, m4_nonzero):
    nc = bass.Bass()
    x_ext = nc.declare_dram_parameter("x", [PER, C, H, H], F32, isOutput=False)
    out_ext = nc.declare_dram_parameter("out", [PER, C, H, H], F32, isOutput=True)

    w1_d = nc.inline_tensor(w1t.astype(mybir.dt.np(BF16)), name="w1s")
    w2_d = nc.inline_tensor(w2t.astype(mybir.dt.np(BF16)), name="w2s")
    ckeys = sorted(consts.keys())
    ccols = []
    cmap = {}
    for k in ckeys:
        cmap[k] = len(ccols)
        ccols.append(consts[k])
    ctab = np.concatenate(ccols, axis=1)  # [128, NC]
    ctab_d = nc.inline_tensor(ctab.astype(np.float32), name="ctab")
    NC_COL = ctab.shape[1]
    coff = {}
    o = 0
    for k in ckeys:
        coff[k] = o
        o += consts[k].shape[1]

    with tile.TileContext(nc) as tc:
        with (
            tc.tile_pool(name="persist", bufs=1) as persist,
            tc.tile_pool(name="big", bufs=1) as big,
            tc.tile_pool(name="stats", bufs=1) as stats,
            tc.tile_pool(name="tiny", bufs=1) as tiny,
            tc.tile_pool(name="p4", bufs=2) as p4pool,
            tc.tile_pool(name="scr", bufs=2) as scr,
            tc.tile_pool(name="psum", bufs=2, space="PSUM") as psum,
            tc.tile_pool(name="dram", bufs=1, space="DRAM") as dram,
        ):
            # ---- constants to SBUF ----
            w1_sb = persist.tile([CM, 2, 9, 64], BF16, tag="w1")
            nc.sync.dma_start(out=w1_sb[:], in_=w1_d[:])
            w2_sb = persist.tile([CM, 9, 128], BF16, tag="w2")
            nc.sync.dma_start(out=w2_sb[:], in_=w2_d[:])
            ct = persist.tile([CM, NC_COL], F32, tag="ctab")
            nc.sync.dma_start(out=ct[:], in_=ctab_d[:])

            def cc(k, j=0):  # const column AP [128, 1]
                return ct[:, coff[k] + j : coff[k] + j + 1]

            eps_sb = persist.tile([CM, 1], F32, tag="eps")
            nc.vector.memset(eps_sb[:], EPS)

            # ---- big buffers ----
            x_sb = [persist.tile([CM, PER, IMG], F32, tag=f"x{g}", name=f"x_sb{g}") for g in range(2)]
            s1 = [big.tile([CM, 2 * MARG + FP], BF16, tag=f"ba{g}", name=f"s1_{g}") for g in range(2)]
            u_c = big.tile([CM, PER, H, H], F32, tag="fa")
            r_c = big.tile([CM, PER, H, H], BF16, tag="fb")

            # zero the sign buffers (padding + margins must be 0)
            nc.gpsimd.memset(s1[0][:], 0.0)
            nc.gpsimd.memset(s1[1][:], 0.0)

            # ---- load x ----
            for g in range(2):
                src = x_ext[:, g * CM:(g + 1) * CM, :, :].rearrange(
                    "n c h w -> c n (h w)"
                )
                nc.sync.dma_start(out=x_sb[g][:], in_=src)

            # ---- helper: per-channel moments -> (E, E2) cols of arbuf ----
            def moments_from_aggr(mv, arbuf, col):
                # mv: [128, 2] (mean, var); write E to col, E2=var+mean^2 to col+1
                sq = tiny.tile([CM, 1], F32)
                nc.vector.tensor_tensor(
                    out=sq[:], in0=mv[:, 0:1], in1=mv[:, 0:1], op=OP.mult
                )
                nc.vector.tensor_tensor(
                    out=arbuf[:, col + 1 : col + 2], in0=mv[:, 1:2], in1=sq[:],
                    op=OP.add,
                )
                nc.vector.tensor_copy(out=arbuf[:, col : col + 1], in_=mv[:, 0:1])

            def bn_stats_over(src_ap_list, tagname):
                # src_ap_list: equal-count APs (2D [p, f] or 3D [p, r, c]
                # with equal c); bn_stats emits one 6-stat per innermost row
                n = len(src_ap_list)
                rows = 1
                if len(src_ap_list[0].shape) == 3:
                    rows = src_ap_list[0].shape[1]
                st = stats.tile([CM, n * rows, 6], F32, tag=f"st_{tagname}")
                for j, ap in enumerate(src_ap_list):
                    nc.vector.bn_stats(out=st[:, j * rows : (j + 1) * rows, :], in_=ap)
                mv = stats.tile([CM, 2], F32, tag=f"mv_{tagname}")
                nc.vector.bn_aggr(out=mv[:], in_=st[:])
                return mv

            # ---- bn1 stats ----
            arbuf1 = stats.tile([CM, 4], F32, tag="ar1")
            for g in range(2):
                xf = x_sb[g].rearrange("p n i -> p (n i)")
                chunks = [xf[:, j * 448 : (j + 1) * 448] for j in range(14)]
                mv = bn_stats_over(chunks, f"x{g}")
                moments_from_aggr(mv, arbuf1, 2 * g)

            # ---- AllReduce helper ----
            def allreduce(arbuf, ncol, tag):
                bin_ = dram.tile([CM, ncol], F32, tag=f"cin_{tag}")
                bout = dram.tile([CM, ncol], F32, tag=f"cout_{tag}", addr_space="Shared")
                nc.gpsimd.dma_start(out=bin_[:], in_=arbuf[:])
                nc.gpsimd.collective_compute(
                    "AllReduce",
                    OP.add,
                    replica_groups=[list(range(NCORES))],
                    ins=[bin_[:]],
                    outs=[bout[:]],
                )
                res = stats.tile([CM, ncol], F32, tag=f"ares_{tag}")
                nc.gpsimd.dma_start(out=res[:], in_=bout[:])
                return res

            ar1 = allreduce(arbuf1, 4, "1")

            # ---- bn1 coefs + sign1 ----
            def rstd_from(ar, col, tagname):
                # returns (mean, rstd) [128,1] tiles from AR cols (E, E2)
                mean = tiny.tile([CM, 1], F32, tag=f"mean_{tagname}")
                nc.vector.tensor_scalar(
                    out=mean[:], in0=ar[:, col : col + 1],
                    scalar1=1.0 / NCORES, scalar2=None, op0=OP.mult,
                )
                e2 = tiny.tile([CM, 1], F32, tag=f"e2_{tagname}")
                nc.vector.tensor_scalar(
                    out=e2[:], in0=ar[:, col + 1 : col + 2],
                    scalar1=1.0 / NCORES, scalar2=None, op0=OP.mult,
                )
                var = tiny.tile([CM, 1], F32, tag=f"var_{tagname}")
                nc.vector.tensor_tensor(out=var[:], in0=mean[:], in1=mean[:], op=OP.mult)
                nc.vector.tensor_tensor(out=var[:], in0=e2[:], in1=var[:], op=OP.subtract)
                std = tiny.tile([CM, 1], F32, tag=f"std_{tagname}")
                nc.scalar.activation(out=std[:], in_=var[:], func=AF.Sqrt, bias=eps_sb[:])
                rstd = tiny.tile([CM, 1], F32, tag=f"rstd_{tagname}")
                nc.vector.reciprocal(out=rstd[:], in_=std[:])
                return mean, rstd

            for g in range(2):
                mean1, rstd1 = rstd_from(ar1, 2 * g, f"bn1_{g}")
                sc = tiny.tile([CM, 1], F32, tag=f"s1c_{g}")
                nc.vector.tensor_tensor(out=sc[:], in0=cc("g1", g), in1=rstd1[:], op=OP.mult)
                bi = tiny.tile([CM, 1], F32, tag=f"s1b_{g}")
                nc.vector.tensor_tensor(out=bi[:], in0=sc[:], in1=mean1[:], op=OP.mult)
                nc.vector.tensor_tensor(out=bi[:], in0=cc("b1m1", g), in1=bi[:], op=OP.subtract)
                # sign1 into padded interior
                dst = s1[g][:, MARG : MARG + FP].rearrange(
                    "p (n r c) -> p n r c", n=PER, r=HP
                )[:, :, 1 : 1 + H, 1 : 1 + H]
                nc.scalar.activation(
                    out=dst, in_=x_sb[g].rearrange("p n (r c) -> p n r c", r=H),
                    func=AF.Sign, bias=bi[:], scale=sc[:],
                )

            # ---- conv1 + interior-only psum eviction ----
            for (nimg, b, st0, ln) in CHUNKS:
                pt = psum.tile([CM, 512], F32, tag="ps1", bufs=4)
                for g in range(2):
                    for k in range(9):
                        nc.tensor.matmul(
                            out=pt[g * 64:(g + 1) * 64, :ln],
                            lhsT=w1_sb[:, g, k, :],
                            rhs=s1[g][:, MARG + st0 + _shift(k) : MARG + st0 + _shift(k) + ln],
                            start=(k == 0),
                            stop=(k == 8),
                        )
                r0, nr, orow = _evict_rows(b)
                src_ap = pt[:, :ln].rearrange("p (r c) -> p r c", c=HP)[
                    :, r0 : r0 + nr, 1 : 1 + H
                ]
                nc.scalar.activation(
                    out=u_c[:, nimg, orow : orow + nr, :], in_=src_ap,
                    func=AF.Copy, bias=0.0, scale=cc("sf1"),
                )
                nc.scalar.activation(
                    out=r_c[:, nimg, orow : orow + nr, :], in_=src_ap,
                    func=AF.Relu, bias=0.0, scale=cc("sf1"),
                )

            # ---- bn2 stats ----
            arbuf2 = stats.tile([CM, 4], F32, tag="ar2")
            uf = u_c.rearrange("p n r c -> p (n r c)")
            mv_u = bn_stats_over(
                [uf[:, j * 448 : (j + 1) * 448] for j in range(14)], "u"
            )
            moments_from_aggr(mv_u, arbuf2, 0)
            rf = r_c.rearrange("p n r c -> p (n r c)")
            mv_r = bn_stats_over(
                [rf[:, j * 448 : (j + 1) * 448] for j in range(14)], "r"
            )
            moments_from_aggr(mv_r, arbuf2, 2)
            ar2 = allreduce(arbuf2, 4, "2")

            # ---- sign2 threshold ----
            # out1 = (1-a1)*r + a1*u ; sign2 = sign(a2n*(out1) + c2n)
            #      = sign(u - theta) given a1>0, a2n>0
            eu = tiny.tile([CM, 1], F32, tag="eu")
            nc.vector.tensor_scalar(out=eu[:], in0=ar2[:, 0:1], scalar1=1.0 / NCORES, scalar2=None, op0=OP.mult)
            eu2 = tiny.tile([CM, 1], F32, tag="eu2")
            nc.vector.tensor_scalar(out=eu2[:], in0=ar2[:, 1:2], scalar1=1.0 / NCORES, scalar2=None, op0=OP.mult)
            er = tiny.tile([CM, 1], F32, tag="er")
            nc.vector.tensor_scalar(out=er[:], in0=ar2[:, 2:3], scalar1=1.0 / NCORES, scalar2=None, op0=OP.mult)
            er2 = tiny.tile([CM, 1], F32, tag="er2")
            nc.vector.tensor_scalar(out=er2[:], in0=ar2[:, 3:4], scalar1=1.0 / NCORES, scalar2=None, op0=OP.mult)

            eo1 = tiny.tile([CM, 1], F32, tag="eo1")
            t_a = tiny.tile([CM, 1], F32, tag="t_a")
            nc.vector.tensor_tensor(out=eo1[:], in0=cc("a1"), in1=eu[:], op=OP.mult)
            nc.vector.tensor_tensor(out=t_a[:], in0=cc("oma1"), in1=er[:], op=OP.mult)
            nc.vector.tensor_tensor(out=eo1[:], in0=eo1[:], in1=t_a[:], op=OP.add)
            eo1sq = tiny.tile([CM, 1], F32, tag="eo1sq")
            nc.vector.tensor_tensor(out=eo1sq[:], in0=cc("a1sq"), in1=eu2[:], op=OP.mult)
            nc.vector.tensor_tensor(out=t_a[:], in0=cc("oma1sq"), in1=er2[:], op=OP.mult)
            nc.vector.tensor_tensor(out=eo1sq[:], in0=eo1sq[:], in1=t_a[:], op=OP.add)
            var2 = tiny.tile([CM, 1], F32, tag="var2")
            nc.vector.tensor_tensor(out=var2[:], in0=eo1[:], in1=eo1[:], op=OP.mult)
            nc.vector.tensor_tensor(out=var2[:], in0=eo1sq[:], in1=var2[:], op=OP.subtract)
            std2 = tiny.tile([CM, 1], F32, tag="std2")
            nc.scalar.activation(out=std2[:], in_=var2[:], func=AF.Sqrt, bias=eps_sb[:])
            rstd2 = tiny.tile([CM, 1], F32, tag="rstd2")
            nc.vector.reciprocal(out=rstd2[:], in_=std2[:])
            a2n = tiny.tile([CM, 1], F32, tag="a2n")
            nc.vector.tensor_tensor(out=a2n[:], in0=cc("g2"), in1=rstd2[:], op=OP.mult)
            c2n = tiny.tile([CM, 1], F32, tag="c2n")
            nc.vector.tensor_tensor(out=c2n[:], in0=a2n[:], in1=eo1[:], op=OP.mult)
            nc.vector.tensor_tensor(out=c2n[:], in0=cc("b2m2"), in1=c2n[:], op=OP.subtract)
            # vstar = -c2n/a2n ; theta = vstar if vstar>=0 else vstar/a1
            inva2n = tiny.tile([CM, 1], F32, tag="inva2n")
            nc.vector.reciprocal(out=inva2n[:], in_=a2n[:])
            vstar = tiny.tile([CM, 1], F32, tag="vstar")
            nc.vector.tensor_tensor(out=vstar[:], in0=c2n[:], in1=inva2n[:], op=OP.mult)
            nc.vector.tensor_scalar(out=vstar[:], in0=vstar[:], scalar1=-1.0, scalar2=None, op0=OP.mult)
            mask = tiny.tile([CM, 1], F32, tag="mask")
            nc.vector.tensor_scalar(out=mask[:], in0=vstar[:], scalar1=0.0, scalar2=None, op0=OP.is_ge)
            th_a = tiny.tile([CM, 1], F32, tag="th_a")
            nc.vector.tensor_tensor(out=th_a[:], in0=mask[:], in1=vstar[:], op=OP.mult)
            onem = tiny.tile([CM, 1], F32, tag="onem")
            nc.vector.tensor_scalar(out=onem[:], in0=mask[:], scalar1=-1.0, scalar2=1.0, op0=OP.mult, op1=OP.add)
            th_b = tiny.tile([CM, 1], F32, tag="th_b")
            nc.vector.tensor_tensor(out=th_b[:], in0=vstar[:], in1=cc("inva1"), op=OP.mult)
            nc.vector.tensor_tensor(out=th_b[:], in0=onem[:], in1=th_b[:], op=OP.mult)
            negth = tiny.tile([CM, 1], F32, tag="negth")
            nc.vector.tensor_tensor(out=negth[:], in0=th_a[:], in1=th_b[:], op=OP.add)
            nc.vector.tensor_scalar(out=negth[:], in0=negth[:], scalar1=-1.0, scalar2=None, op0=OP.mult)

            # ---- sign2 (natural layout: partitions 0..63 = group1 input
            # channels, 64..127 = group2 -- matches conv2 weight layout) ----
            s2a = big.tile([CM, 2 * MARG + FP], BF16, tag="ba0")
            nc.gpsimd.memset(s2a[:], 0.0)
            dsta = s2a[:, MARG : MARG + FP].rearrange(
                "p (n r c) -> p n r c", n=PER, r=HP
            )[:, :, 1 : 1 + H, 1 : 1 + H]
            nc.scalar.activation(out=dsta, in_=u_c[:], func=AF.Sign, bias=negth[:], scale=1.0)

            # ---- conv2: per chunk, 9 taps x 2 groups; the groups run in
            # disjoint PE row-groups (rows 0-63 / 64-127) into separate
            # PSUM banks, so their matmul streams overlap ----
            u2 = [
                big.tile([CM, PER, H, H], F32, tag=t, name=f"u2_{t}")
                for t in ("fa", "fb")
            ]
            for (nimg, b, st0, ln) in CHUNKS:
                pt2 = [psum.tile([CM, 512], F32, tag=f"ps2_{g}", name=f"pt2_{g}") for g in range(2)]
                for k in range(9):
                    for g in range(2):
                        sl = MARG + st0 + _shift(k)
                        nc.tensor.matmul(
                            out=pt2[g][:, :ln],
                            lhsT=w2_sb[g * 64:(g + 1) * 64, k, :],
                            rhs=s2a[g * 64:(g + 1) * 64, sl : sl + ln],
                            start=(k == 0),
                            stop=(k == 8),
                            tile_position=(g * 64, 0),
                        )
                r0, nr, orow = _evict_rows(b)
                for g in range(2):
                    nc.scalar.activation(
                        out=u2[g][:, nimg, orow : orow + nr, :],
                        in_=pt2[g][:, :ln].rearrange("p (r c) -> p r c", c=HP)[
                            :, r0 : r0 + nr, 1 : 1 + H
                        ],
                        func=AF.Copy, bias=0.0, scale=1.0,
                    )

            # ---- bn3 stats ----
            # u2 via bn_stats; r2 = relu(u2) via two whole-tensor ACT
            # passes with accumulate output (r2 scratch reused in place)
            arbuf3 = stats.tile([CM, 8], F32, tag="ar3")
            for t in range(2):
                u2f = u2[t].rearrange("p n r c -> p (n r c)")
                mv3 = bn_stats_over(
                    [u2f[:, j * 448 : (j + 1) * 448] for j in range(14)],
                    f"u2_{t}",
                )
                moments_from_aggr(mv3, arbuf3, 4 * t)
                r2s = big.tile([CM, F], BF16, tag="ba1", name=f"r2s_{t}")
                sum_r = stats.tile([CM, 1], F32, tag=f"sum_r_{t}")
                nc.scalar.activation(
                    out=r2s[:], in_=u2f, func=AF.Relu, accum_out=sum_r[:]
                )
                sum_r2 = stats.tile([CM, 1], F32, tag=f"sum_r2_{t}")
                nc.scalar.activation(
                    out=r2s[:], in_=r2s[:], func=AF.Square, accum_out=sum_r2[:]
                )
                nc.vector.tensor_scalar(
                    out=arbuf3[:, 4 * t + 2 : 4 * t + 3], in0=sum_r[:],
                    scalar1=1.0 / F, scalar2=None, op0=OP.mult,
                )
                nc.vector.tensor_scalar(
                    out=arbuf3[:, 4 * t + 3 : 4 * t + 4], in0=sum_r2[:],
                    scalar1=1.0 / F, scalar2=None, op0=OP.mult,
                )
            ar3 = allreduce(arbuf3, 8, "3")

            # ---- bn3 coefs + phase4 ----
            for t in range(2):
                eu_ = tiny.tile([CM, 1], F32, tag=f"p4eu_{t}")
                nc.vector.tensor_scalar(out=eu_[:], in0=ar3[:, 4 * t : 4 * t + 1], scalar1=1.0 / NCORES, scalar2=None, op0=OP.mult)
                eu2_ = tiny.tile([CM, 1], F32, tag=f"p4eu2_{t}")
                nc.vector.tensor_scalar(out=eu2_[:], in0=ar3[:, 4 * t + 1 : 4 * t + 2], scalar1=1.0 / NCORES, scalar2=None, op0=OP.mult)
                er_ = tiny.tile([CM, 1], F32, tag=f"p4er_{t}")
                nc.vector.tensor_scalar(out=er_[:], in0=ar3[:, 4 * t + 2 : 4 * t + 3], scalar1=1.0 / NCORES, scalar2=None, op0=OP.mult)
                er2_ = tiny.tile([CM, 1], F32, tag=f"p4er2_{t}")
                nc.vector.tensor_scalar(out=er2_[:], in0=ar3[:, 4 * t + 3 : 4 * t + 4], scalar1=1.0 / NCORES, scalar2=None, op0=OP.mult)

                eo2 = tiny.tile([CM, 1], F32, tag=f"eo2_{t}")
                tb = tiny.tile([CM, 1], F32, tag=f"tb_{t}")
                nc.vector.tensor_tensor(out=eo2[:], in0=cc("casf", t), in1=eu_[:], op=OP.mult)
                nc.vector.tensor_tensor(out=tb[:], in0=cc("cbsf", t), in1=er_[:], op=OP.mult)
                nc.vector.tensor_tensor(out=eo2[:], in0=eo2[:], in1=tb[:], op=OP.add)
                eo2sq = tiny.tile([CM, 1], F32, tag=f"eo2sq_{t}")
                nc.vector.tensor_tensor(out=eo2sq[:], in0=cc("casq", t), in1=eu2_[:], op=OP.mult)
                nc.vector.tensor_tensor(out=tb[:], in0=cc("cbsq", t), in1=er2_[:], op=OP.mult)
                nc.vector.tensor_tensor(out=eo2sq[:], in0=eo2sq[:], in1=tb[:], op=OP.add)
                var3 = tiny.tile([CM, 1], F32, tag=f"var3_{t}")
                nc.vector.tensor_tensor(out=var3[:], in0=eo2[:], in1=eo2[:], op=OP.mult)
                nc.vector.tensor_tensor(out=var3[:], in0=eo2sq[:], in1=var3[:], op=OP.subtract)
                std3 = tiny.tile([CM, 1], F32, tag=f"std3_{t}")
                nc.scalar.activation(out=std3[:], in_=var3[:], func=AF.Sqrt, bias=eps_sb[:])
                a3n = tiny.tile([CM, 1], F32, tag=f"a3n_{t}")
                nc.vector.reciprocal(out=a3n[:], in_=std3[:])
                nc.vector.tensor_tensor(out=a3n[:], in0=cc("g3", t), in1=a3n[:], op=OP.mult)
                rc = tiny.tile([CM, 1], F32, tag=f"rc_{t}")
                nc.vector.tensor_tensor(out=rc[:], in0=a3n[:], in1=eo2[:], op=OP.mult)
                nc.vector.tensor_tensor(out=rc[:], in0=cc("b3m3", t), in1=rc[:], op=OP.subtract)
                p_ = tiny.tile([CM, 1], F32, tag=f"p_{t}")
                nc.vector.tensor_tensor(out=p_[:], in0=a3n[:], in1=cc("h1p", t), op=OP.mult)
                q_ = tiny.tile([CM, 1], F32, tag=f"q_{t}")
                nc.vector.tensor_tensor(out=q_[:], in0=a3n[:], in1=cc("h1m", t), op=OP.mult)

                # phase4 slices: per image, 2 blocks of 28 rows (all flat)
                u2f2 = u2[t].rearrange("p n r c -> p (n r c)")
                xf2 = x_sb[t].rearrange("p n i -> p (n i)")
                outv = out_ext[:, t * CM:(t + 1) * CM, :, :].rearrange(
                    "n c h w -> c n (h w)"
                )
                SL = 28 * H
                for sli in range(PER * 2):
                    s0 = sli * SL
                    u2s = u2f2[:, s0 : s0 + SL]
                    xs = xf2[:, s0 : s0 + SL]
                    h_sl = p4pool.tile([CM, SL], F32, tag="p4h")
                    nc.scalar.activation(out=h_sl[:], in_=u2s, func=AF.Abs, bias=0.0, scale=q_[:])
                    z1 = p4pool.tile([CM, SL], F32, tag="p4z1")
                    nc.vector.scalar_tensor_tensor(
                        out=z1[:], in0=u2s, scalar=p_[:], in1=h_sl[:],
                        op0=OP.mult, op1=OP.add,
                    )
                    z = p4pool.tile([CM, SL], F32, tag="p4z")
                    nc.vector.scalar_tensor_tensor(
                        out=z[:], in0=xs, scalar=rc[:], in1=z1[:],
                        op0=OP.add, op1=OP.add,
                    )
                    gz = p4pool.tile([CM, SL], F32, tag="p4g")
                    nc.scalar.activation(out=gz[:], in_=z[:], func=AF.Abs, bias=0.0, scale=cc("B3", t))
                    o_sl = p4pool.tile([CM, SL], F32, tag="p4h", name="o_sl")
                    nc.vector.scalar_tensor_tensor(
                        out=o_sl[:], in0=z[:], scalar=cc("A3", t), in1=gz[:],
                        op0=OP.mult, op1=OP.add,
                    )
                    if m4_nonzero:
                        nc.vector.tensor_scalar(
                            out=o_sl[:], in0=o_sl[:], scalar1=cc("m4", t),
                            scalar2=None, op0=OP.add,
                        )
                    nc.sync.dma_start(
                        out=outv[:, sli // 2, (sli % 2) * SL : (sli % 2) * SL + SL],
                        in_=o_sl[:],
                    )
    return nc


def run(inputs, trace=False, trace_kwargs=None):
    _apply_tile_patch()
    from concourse.bass_utils import run_bass_kernel_spmd

    x = np.ascontiguousarray(inputs["x"], dtype=np.float32)  # [16, 256, 56, 56]
    w1t, w2t, consts, m4_nonzero = _host_consts(inputs)
    nc = _build(w1t, w2t, consts, m4_nonzero)
    _split_excess_waits(nc)

    in_maps = [
        {"x": np.ascontiguousarray(x[c * PER:(c + 1) * PER])} for c in range(NCORES)
    ]
    res = run_bass_kernel_spmd(
        nc, in_maps, list(range(NCORES)), trace=trace, **(trace_kwargs or {})
    )
    out = np.concatenate([res.results[c]["out"] for c in range(NCORES)], axis=0)
    return out.astype(np.float32), res


def kernel(**inputs):
    out, _ = run(inputs)
    return out
